# revision 3
# baseline (speedup 1.0000x reference)
"""BMC loss (InfoNCE-style MVN loss) on 8 trn2 NeuronCores — fp8 DoubleRow.

loss = mean_i( LSE_j(l_ij) - l_ii ) * 2*sigma^2,  l_ij = (p_i.t_j - 0.5|t_j|^2)/nv
(per-row constants -0.5|p_i|^2/nv and the log-norm cancel between LSE and diag)

Device work per core (slab = 1024 pred rows, all 8192 targets):
  v'_ij = cross8_ij + (hi_j + lo_j)           where cross8 = fp8(p).fp8(t),
      hi/lo = fp8 high/low split of -(0.5|t_j|^2 - T0), T0 = host constant
  C_i   = max_{j in 16Z} v'_ij + DELTA        (stride-16 subsampled row max)
  s_i   = sum_j exp((v'_ij - C_i)/nv)
returning C_i and s_i. Host (exact, f64) finishes:
  rowloss_i = (C_i - T0 - v_ii)/nv + ln s_i,  v_ii = p_i.t_i - 0.5|t_i|^2
  loss = 2*nv*mean(rowloss)

Engine plan (per core):
  PE  : one fp8 DoubleRow matmul per 512-col tile = full K=256 contraction at
        0.5 cyc/row, plus a K=1 DoubleRow rank-1 (ones x [hi;lo]) that folds
        the -0.5|t|^2 column term straight into PSUM  (~70k cycles, ~29us)
  ACT : single pass - Exp reads PSUM directly with scale=1/nv and per-row
        bias=-C_i/nv, accum_out produces row sums (~55us  <- bottleneck)
  DVE : only the subsampled-max pre-pass + tiny scalar fixups (~6us)
  (no DVE evacuation pass, no on-device t2/diag/log: host does those)

fp8 error budget (validated on the fixed-seed data): loss bias ~ -0.016
absolute vs tolerance ~2.7; max exp arg ~28 vs f32 overflow at 88.
"""

import numpy as np

B = 8192
D = 256
NCORES = 8
P = 128
JT = 512        # matmul moving free dim (one PSUM bank)
GROUP = 2048    # PSUM tile cols (4 banks); 2 tiles fill all of PSUM
SUBSTRIDE = 16
DELTA = 6.0


def _build(b=B, slab=B // NCORES, reps=1):
    import concourse.bass as bass
    import concourse.mybir as mybir
    import concourse.tile as tile
    from concourse import bacc
    from contextlib import ExitStack

    f32 = mybir.dt.float32
    f8 = mybir.dt.float8e4
    DR = mybir.MatmulPerfMode.DoubleRow

    kc_n = D // P            # 2 k-tiles of 128
    it_n = slab // P         # 8 i-tiles per core
    ng = b // GROUP          # 4 PSUM groups per i-tile row
    jg = GROUP // JT         # 4 j-tiles per group
    sub = b // SUBSTRIDE     # 512 subsampled columns

    nc = bacc.Bacc("TRN2", target_bir_lowering=False, debug=False)
    predT8 = nc.dram_tensor("predT8", [P, kc_n, slab], f8, kind="ExternalInput")
    targetT8 = nc.dram_tensor("targetT8", [P, kc_n, b], f8, kind="ExternalInput")
    tsub8 = nc.dram_tensor("tsub8", [P, kc_n, sub], f8, kind="ExternalInput")
    t2hl = nc.dram_tensor("t2hl", [1, kc_n, b], f8, kind="ExternalInput")
    t2subhl = nc.dram_tensor("t2subhl", [1, kc_n, sub], f8, kind="ExternalInput")
    ones8 = nc.dram_tensor("ones8", [1, kc_n, P], f8, kind="ExternalInput")
    sigma = nc.dram_tensor("sigma", [1, 1], f32, kind="ExternalInput")
    cs_out = nc.dram_tensor("cs", [P, 2 * it_n], f32, kind="ExternalOutput")

    with ExitStack() as ctx:
        tc = ctx.enter_context(tile.TileContext(nc))
        singles = ctx.enter_context(tc.tile_pool(name="singles", bufs=1))
        psum = ctx.enter_context(tc.tile_pool(name="psum", bufs=2, space="PSUM"))
        scratch = ctx.enter_context(tc.tile_pool(name="scratch", bufs=2))
        smalls = ctx.enter_context(tc.tile_pool(name="smalls", bufs=2))

        # ---- input DMA, round-robin across engine DGE queues ----
        issuers = [nc.sync, nc.scalar, nc.gpsimd]
        rr = [0]

        def dma(out, in_):
            eng = issuers[rr[0] % len(issuers)]
            rr[0] += 1
            eng.dma_start(out=out, in_=in_)

        predT_sb = singles.tile([P, kc_n, slab], f8)
        dma(predT_sb, predT8[:, :, :])
        tsub_sb = singles.tile([P, kc_n, sub], f8)
        dma(tsub_sb, tsub8[:, :, :])
        t2subhl_sb = singles.tile([1, kc_n, sub], f8)
        dma(t2subhl_sb, t2subhl[:, :, :])
        t2hl_sb = singles.tile([1, kc_n, b], f8)
        dma(t2hl_sb, t2hl[:, :, :])
        ones_sb = singles.tile([1, kc_n, P], f8)
        dma(ones_sb, ones8[:, :, :])
        sigma_sb = singles.tile([P, 1], f32)
        nc.gpsimd.dma_start(
            out=sigma_sb,
            in_=bass.AP(
                tensor=sigma[0:1, :].tensor,
                offset=sigma[0:1, :].offset,
                ap=[[0, P]] + list(sigma[0:1, :].ap[1:]),
            ),
        )
        targetT_sb = singles.tile([P, kc_n, b], f8)
        seg = 2048
        for s in range(b // seg):
            dma(targetT_sb[:, :, s * seg : (s + 1) * seg],
                targetT8[:, :, s * seg : (s + 1) * seg])

        nv128 = singles.tile([P, 1], f32)
        nc.vector.tensor_tensor(nv128, sigma_sb, sigma_sb, mybir.AluOpType.mult)
        inv128 = singles.tile([P, 1], f32)
        nc.vector.reciprocal(inv128, nv128)
        neg_inv128 = singles.tile([P, 1], f32)
        nc.vector.tensor_scalar_mul(neg_inv128, inv128, -1.0)
        # prime the ACT Exp table while DMAs run (implicit table load ~1.3us)
        warm = singles.tile([P, 1], f32)
        nc.scalar.activation(out=warm, in_=nv128,
                             func=mybir.ActivationFunctionType.Exp)

        for _rep in range(reps):
            mraw = singles.tile([P, it_n], f32)
            C_all = singles.tile([P, it_n], f32)
            bias_all = singles.tile([P, it_n], f32)
            s_parts = singles.tile([P, it_n, ng], f32)
            s_all = singles.tile([P, it_n, 1], f32)

            # ---- pre-pass: subsampled row max over v' = cross8 - t2' ----
            # 4 i-tiles of [128, 512] per PSUM tile; one batched DVE max each
            for half in range(it_n // 4):
                ps_pre = psum.tile([P, 4, JT], f32, tag="mm")
                for q in range(4):
                    t = half * 4 + q
                    nc.tensor.matmul(
                        out=ps_pre[:, q, :],
                        lhsT=predT_sb[:, :, t * P : (t + 1) * P],
                        rhs=tsub_sb,
                        start=True, stop=False, perf_mode=DR,
                    )
                    nc.tensor.matmul(
                        out=ps_pre[:, q, :],
                        lhsT=ones_sb,
                        rhs=t2subhl_sb,
                        start=False, stop=True, perf_mode=DR,
                    )
                nc.vector.tensor_reduce(
                    out=mraw[:, half * 4 : (half + 1) * 4],
                    in_=ps_pre,
                    axis=mybir.AxisListType.X,
                    op=mybir.AluOpType.max,
                )
            nc.vector.tensor_scalar_add(C_all, mraw, DELTA)
            nc.vector.tensor_scalar_mul(bias_all, C_all, neg_inv128)

            # ---- main: per (i-tile, 2048-col group): 8 matmuls -> 1 Exp ----
            for t in range(it_n):
                for g in range(ng):
                    ps = psum.tile([P, GROUP], f32, tag="mm")
                    for jj in range(jg):
                        j = g * jg + jj
                        nc.tensor.matmul(
                            out=ps[:, jj * JT : (jj + 1) * JT],
                            lhsT=predT_sb[:, :, t * P : (t + 1) * P],
                            rhs=targetT_sb[:, :, j * JT : (j + 1) * JT],
                            start=True, stop=False, perf_mode=DR,
                        )
                        nc.tensor.matmul(
                            out=ps[:, jj * JT : (jj + 1) * JT],
                            lhsT=ones_sb,
                            rhs=t2hl_sb[:, :, j * JT : (j + 1) * JT],
                            start=False, stop=True, perf_mode=DR,
                        )
                    ex = scratch.tile([P, GROUP], f32, tag="ex")
                    nc.scalar.activation(
                        out=ex,
                        in_=ps,
                        func=mybir.ActivationFunctionType.Exp,
                        bias=bias_all[:, t : t + 1],
                        scale=inv128,
                        accum_out=s_parts[:, t, g : g + 1],
                    )

            nc.vector.tensor_reduce(
                out=s_all,
                in_=s_parts,
                axis=mybir.AxisListType.X,
                op=mybir.AluOpType.add,
            )
            nc.sync.dma_start(out=cs_out[:, 0:it_n], in_=C_all)
            nc.sync.dma_start(out=cs_out[:, it_n : 2 * it_n], in_=s_all)

    nc.compile()
    return nc


_NC = None
_TRACE = False
_LAST_RESULT = [None]


def _f8(x):
    import ml_dtypes

    return np.asarray(x, dtype=np.float32).astype(ml_dtypes.float8_e4m3)


def _make_in_maps(pred, target, sig):
    """Shard + quantize. Returns (in_maps, host_ctx) where host_ctx has the
    exact f64 quantities the host needs to finish the loss."""
    import ml_dtypes

    slab = B // NCORES
    p64 = pred.astype(np.float64)
    t64 = target.astype(np.float64)
    t2h = 0.5 * np.sum(t64 * t64, axis=1)            # [B]
    T0 = float(np.median(t2h))
    t2c = -(t2h - T0)                                # value to add to cross
    hi = _f8(t2c)
    lo = _f8(t2c - hi.astype(np.float64))
    t2hl_full = np.stack([hi, lo], axis=0)[None]     # [1, 2, B] fp8
    v_ii = np.sum(p64 * t64, axis=1) - t2h           # [B] exact diag

    pred8 = _f8(pred)    # [B, D]
    target8 = _f8(target)
    # [D, n] -> [128, 2, n] (k within tile, k-tile, column)
    targetT = np.ascontiguousarray(
        target8.T.reshape(2, P, B).transpose(1, 0, 2))
    tsub = np.ascontiguousarray(targetT[:, :, ::SUBSTRIDE])
    t2subhl = np.ascontiguousarray(t2hl_full[:, :, ::SUBSTRIDE])
    ones8 = np.ones((1, 2, P), dtype=ml_dtypes.float8_e4m3)

    in_maps = []
    for c in range(NCORES):
        sl = slice(c * slab, (c + 1) * slab)
        predT = np.ascontiguousarray(
            pred8[sl].T.reshape(2, P, slab).transpose(1, 0, 2))
        in_maps.append(
            {
                "predT8": predT,
                "targetT8": targetT,
                "tsub8": tsub,
                "t2hl": t2hl_full,
                "t2subhl": t2subhl,
                "ones8": ones8,
                "sigma": sig,
            }
        )
    return in_maps, {"T0": T0, "v_ii": v_ii}


def kernel(pred, target, noise_sigma):
    global _NC
    from concourse.bass_utils import run_bass_kernel_spmd

    pred = np.ascontiguousarray(np.asarray(pred, dtype=np.float32))
    target = np.ascontiguousarray(np.asarray(target, dtype=np.float32))
    sig = np.asarray(noise_sigma, dtype=np.float32).reshape(1, 1)

    if _NC is None:
        _NC = _build()

    in_maps, hc = _make_in_maps(pred, target, sig)

    kw = {}
    if _TRACE:
        kw = dict(trace=True, stitch_traces=False)
    res = run_bass_kernel_spmd(_NC, in_maps, core_ids=list(range(NCORES)), **kw)
    _LAST_RESULT[0] = res

    slab = B // NCORES
    it_n = slab // P
    nv = np.float64(sig[0, 0]) ** 2
    total = 0.0
    for c, r in enumerate(res.results):
        cs = r["cs"].astype(np.float64)        # [128, 16]
        C = cs[:, :it_n]                       # [p, t] -> row c*slab + t*128 + p
        S = cs[:, it_n:]
        rows = (c * slab + np.arange(it_n)[None, :] * P
                + np.arange(P)[:, None])       # [p, t] global row ids
        v_ii = hc["v_ii"][rows]
        rowloss = (C - hc["T0"] - v_ii) / nv + np.log(S)
        total += rowloss.sum()
    loss = 2.0 * nv * (total / B)
    return np.asarray(loss, dtype=np.float32)


# revision 17
# speedup vs baseline: 5199.9632x; 5199.9632x over previous
"""BMC loss (InfoNCE-style MVN loss) on 8 trn2 NeuronCores — fp8 DoubleRow.

loss = mean_i( LSE_j(l_ij) - l_ii ) * 2*sigma^2,  l_ij = (p_i.t_j - 0.5|t_j|^2)/nv
(per-row constants -0.5|p_i|^2/nv and the log-norm cancel between LSE and diag)

Device work per core (slab = 1024 pred rows, all 8192 targets):
  v'_ij = cross8_ij + t2b_j            cross8 = fp8(p).fp8(t) via DoubleRow,
      t2b = bf16(-(0.5|t_j|^2 - T0)) added into PSUM by a K=1 rank-1 matmul,
      T0 = host median offset
  C_i   = max_{j in 32Z} v'_ij + DELTA (stride-32 subsampled row max pre-pass)
  s_i   = sum_j exp((v'_ij - C_i)/nv)  (ACT reads PSUM, accum_out row sums)
returning C_i and s_i. Host (exact, f64) finishes:
  rowloss_i = (C_i - T0 - v_ii)/nv + ln s_i,  v_ii = p_i.t_i - 0.5|t_i|^2
  loss = 2*nv*mean(rowloss)

Engine plan per core, per half-row phase (8 PSUM banks = 4096 of 8192 cols):
  PE : [load ones_bf16 weights once] 8x K=1 bf16 rank-1 (start) writes -t2
       [load pred i-tile fp8 DR weights once] 8x K=256 DoubleRow cross (stop)
       -> weight thrash avoided: 2 LDWEIGHTS per 4096 cols, not per 512.
  ACT: 2x Exp over [128, 2048] PSUM groups, scale=1/nv, bias=-C_i/nv,
       accum_out -> s.  ACT is the bottleneck (~64us chain incl 352-cyc
       fills + 279ns accumulator reads; exp throughput 1 elem/lane/cycle).
  DVE: subsampled-max pre-pass + tiny fixups only.
Host: fp8/bf16 quantization, exact diag, final log/mean in f64.

fp8 error budget (validated on the fixed-seed data): loss bias ~ -0.02
absolute vs tolerance ~2.7; max exp arg ~45 vs f32 overflow at 88.
"""

import numpy as np

B = 8192
D = 256
NCORES = 8
P = 128
JT = 512        # matmul moving free dim (one PSUM bank)
GROUP = 2048    # ACT instruction span (4 banks)
PHASE = 4096    # PE weight-batch span (8 banks = full PSUM)
SUBSTRIDE = 32
DELTA = 6.0


def _build(b=B, slab=B // NCORES, reps=1, variant="full"):
    import concourse.bass as bass
    import concourse.mybir as mybir
    import concourse.tile as tile
    from concourse import bacc
    from contextlib import ExitStack

    f32 = mybir.dt.float32
    f8 = mybir.dt.float8e4
    bf16 = mybir.dt.bfloat16
    DR = mybir.MatmulPerfMode.DoubleRow

    kc_n = D // P            # 2 k-tiles of 128
    it_n = slab // P         # 8 i-tiles per core
    nph = b // PHASE         # 2 phases per i-tile row
    jg = PHASE // JT         # 8 j-tiles per phase
    sub = b // SUBSTRIDE     # 256 subsampled columns

    nc = bacc.Bacc("TRN2", target_bir_lowering=False, debug=False)
    predT8 = nc.dram_tensor("predT8", [P, kc_n, slab], f8, kind="ExternalInput")
    dpredT8 = nc.dram_tensor("dpredT8", [P, kc_n, slab], f8, kind="ExternalInput")
    targetT8 = nc.dram_tensor("targetT8", [P, kc_n, b], f8, kind="ExternalInput")
    tsub8 = nc.dram_tensor("tsub8", [P, kc_n, sub], f8, kind="ExternalInput")
    t2b = nc.dram_tensor("t2b", [1, b], bf16, kind="ExternalInput")
    t2subb = nc.dram_tensor("t2subb", [1, sub], bf16, kind="ExternalInput")
    onesb = nc.dram_tensor("onesb", [1, P], bf16, kind="ExternalInput")
    sigma = nc.dram_tensor("sigma", [1, 1], f32, kind="ExternalInput")
    cs_out = nc.dram_tensor("cs", [P, 2 * it_n], f32, kind="ExternalOutput")

    with ExitStack() as ctx:
        tc = ctx.enter_context(tile.TileContext(nc))
        singles = ctx.enter_context(tc.tile_pool(name="singles", bufs=1))
        psum = ctx.enter_context(tc.tile_pool(name="psum", bufs=2, space="PSUM"))
        scratch = ctx.enter_context(tc.tile_pool(name="scratch", bufs=2))

        # ---- input DMA, round-robin across engine DGE queues ----
        issuers = [nc.sync, nc.scalar, nc.gpsimd]
        rr = [0]

        def dma(out, in_):
            eng = issuers[rr[0] % len(issuers)]
            rr[0] += 1
            eng.dma_start(out=out, in_=in_)

        predT_sb = singles.tile([P, kc_n, slab], f8)
        dma(predT_sb, predT8[:, :, :])
        dpredT_sb = singles.tile([P, kc_n, slab], f8)
        dma(dpredT_sb, dpredT8[:, :, :])
        tsub_sb = singles.tile([P, kc_n, sub], f8)
        dma(tsub_sb, tsub8[:, :, :])
        t2subb_sb = singles.tile([1, sub], bf16)
        dma(t2subb_sb, t2subb[:, :])
        t2b_sb = singles.tile([1, b], bf16)
        dma(t2b_sb, t2b[:, :])
        ones_sb = singles.tile([1, P], bf16)
        dma(ones_sb, onesb[:, :])
        sigma_sb = singles.tile([P, 1], f32)
        nc.gpsimd.dma_start(
            out=sigma_sb,
            in_=bass.AP(
                tensor=sigma[0:1, :].tensor,
                offset=sigma[0:1, :].offset,
                ap=[[0, P]] + list(sigma[0:1, :].ap[1:]),
            ),
        )
        targetT_sb = singles.tile([P, kc_n, b], f8)
        seg = 2048
        for s in range(b // seg):
            dma(targetT_sb[:, :, s * seg : (s + 1) * seg],
                targetT8[:, :, s * seg : (s + 1) * seg])

        nv128 = singles.tile([P, 1], f32)
        nc.vector.tensor_tensor(nv128, sigma_sb, sigma_sb, mybir.AluOpType.mult)
        inv128 = singles.tile([P, 1], f32)
        nc.vector.reciprocal(inv128, nv128)
        neg_inv128 = singles.tile([P, 1], f32)
        nc.vector.tensor_scalar_mul(neg_inv128, inv128, -1.0)
        # prime the ACT Exp table while DMAs run (implicit table load ~2.7us)
        warm = singles.tile([P, 1], f32)
        nc.scalar.activation(out=warm, in_=nv128,
                             func=mybir.ActivationFunctionType.Exp)

        for _rep in range(reps):
            mraw = singles.tile([P, it_n], f32)
            C_all = singles.tile([P, it_n], f32)
            bias_all = singles.tile([P, it_n], f32)
            if variant != "noexp":
                s_parts = singles.tile([P, it_n, b // GROUP], f32)
                s_all = singles.tile([P, it_n, 1], f32)

            # ---- pre-pass: subsampled row max over v' = cross8 + t2b ----
            # 4 i-tiles of [128, 256] per PSUM tile; rank-1s batched first
            # (one ones-weight load), then the 4 crosses (one LDW each).
            for half in range(it_n // 4):
                ps_pre = psum.tile([P, 4, sub], f32, tag="mm")
                for q in range(4):
                    nc.tensor.matmul(
                        out=ps_pre[:, q, :],
                        lhsT=ones_sb,
                        rhs=t2subb_sb,
                        start=True, stop=False,
                    )
                for q in range(4):
                    t = half * 4 + q
                    nc.tensor.matmul(
                        out=ps_pre[:, q, :],
                        lhsT=predT_sb[:, :, t * P : (t + 1) * P],
                        rhs=tsub_sb,
                        start=False, stop=True, perf_mode=DR,
                    )
                nc.vector.tensor_reduce(
                    out=mraw[:, half * 4 : (half + 1) * 4],
                    in_=ps_pre,
                    axis=mybir.AxisListType.X,
                    op=mybir.AluOpType.max,
                )
            nc.vector.tensor_scalar_add(C_all, mraw, DELTA)
            nc.vector.tensor_scalar_mul(bias_all, C_all, neg_inv128)

            # ---- main loop: j-outer delta chain. Each 4096-col j-phase
            # stays resident in PSUM across all 8 i-tiles: t=0 writes
            # -t2 (rank-1 batch) + cross(pred_0); t>0 adds
            # dpred_t = fp8(pred_t) - fp8(pred_{t-1}) against target,
            # morphing cross(t-1) -> cross(t) in place. One DoubleRow
            # weight load per (phase, t) instead of per 512-col tile. ----
            gpp = PHASE // GROUP  # psum tiles per phase
            jpg = GROUP // JT     # j-tiles per psum tile
            for ph in range(nph):
                pss = []
                for _g in range(gpp):
                    ps_g = psum.tile([P, GROUP], f32, tag="mm")
                    pss.append(ps_g)
                for t in range(it_n):
                    if variant == "nomm" and t == 0:
                        for g in range(gpp):
                            nc.tensor.matmul(
                                out=pss[g][:, 0:JT],
                                lhsT=dpredT_sb[:, :, 0:P],
                                rhs=targetT_sb[:, :, 0:JT],
                                start=True, stop=True, perf_mode=DR,
                            )
                    if variant != "nomm":
                        if t == 0:
                            for g in range(gpp):
                                for jj in range(jpg):
                                    j0 = ph * PHASE + g * GROUP + jj * JT
                                    nc.tensor.matmul(
                                        out=pss[g][:, jj * JT : (jj + 1) * JT],
                                        lhsT=ones_sb,
                                        rhs=t2b_sb[:, j0 : j0 + JT],
                                        start=True, stop=False,
                                    )
                        for g in range(gpp):
                            for jj in range(jpg):
                                j0 = ph * PHASE + g * GROUP + jj * JT
                                o = pss[g][:, jj * JT : (jj + 1) * JT]
                                if variant == "plaindelta":
                                    for kc in range(kc_n):
                                        nc.tensor.matmul(
                                            out=o,
                                            lhsT=dpredT_sb[
                                                :, kc, t * P : (t + 1) * P],
                                            rhs=targetT_sb[
                                                :, kc, j0 : j0 + JT],
                                            start=False,
                                            stop=(kc == kc_n - 1),
                                            skip_group_check=True,
                                        )
                                else:
                                    nc.tensor.matmul(
                                        out=o,
                                        lhsT=dpredT_sb[:, :, t * P : (t + 1) * P],
                                        rhs=targetT_sb[:, :, j0 : j0 + JT],
                                        start=False, stop=True, perf_mode=DR,
                                        skip_group_check=(t > 0),
                                    )
                    if variant != "noexp":
                        for g in range(gpp):
                            ex = scratch.tile([P, GROUP], f32, tag="ex")
                            nc.scalar.activation(
                                out=ex,
                                in_=pss[g],
                                func=mybir.ActivationFunctionType.Exp,
                                bias=bias_all[:, t : t + 1],
                                scale=inv128,
                                accum_out=s_parts[
                                    :, t, ph * gpp + g : ph * gpp + g + 1],
                            )

            if variant != "noexp":
                nc.vector.tensor_reduce(
                    out=s_all,
                    in_=s_parts,
                    axis=mybir.AxisListType.X,
                    op=mybir.AluOpType.add,
                )
                nc.sync.dma_start(out=cs_out[:, it_n : 2 * it_n], in_=s_all)
            nc.sync.dma_start(out=cs_out[:, 0:it_n], in_=C_all)

    nc.compile()
    return nc


_NC = None
_TRACE = False
_LAST_RESULT = [None]


def _f8(x):
    import ml_dtypes

    return np.asarray(x, dtype=np.float32).astype(ml_dtypes.float8_e4m3)


def _make_in_maps(pred, target, sig):
    """Shard + quantize. Returns (in_maps, host_ctx) where host_ctx has the
    exact f64 quantities the host needs to finish the loss."""
    import ml_dtypes

    slab = B // NCORES
    p64 = pred.astype(np.float64)
    t64 = target.astype(np.float64)
    t2h = 0.5 * np.sum(t64 * t64, axis=1)            # [B]
    T0 = float(np.median(t2h))
    t2bf = (-(t2h - T0)).astype(ml_dtypes.bfloat16)[None]  # [1, B]
    v_ii = np.sum(p64 * t64, axis=1) - t2h           # [B] exact diag

    pred8 = _f8(pred)    # [B, D]
    target8 = _f8(target)
    # [D, n] -> [128, 2, n] (k within tile, k-tile, column)
    targetT = np.ascontiguousarray(
        target8.T.reshape(2, P, B).transpose(1, 0, 2))
    tsub = np.ascontiguousarray(targetT[:, :, ::SUBSTRIDE])
    t2subb = np.ascontiguousarray(t2bf[:, ::SUBSTRIDE])
    onesb = np.ones((1, P), dtype=ml_dtypes.bfloat16)

    in_maps = []
    for c in range(NCORES):
        sl = slice(c * slab, (c + 1) * slab)
        p8c = pred8[sl].astype(np.float32)           # [slab, D]
        # delta tiles: row block 0 as-is; block t holds fp8(p8_t - p8_{t-1})
        dp = p8c.copy()
        dp[P:] = p8c[P:] - p8c[:-P]
        dp8 = _f8(dp).astype(np.float32)
        predT = np.ascontiguousarray(
            pred8[sl].T.reshape(2, P, slab).transpose(1, 0, 2))
        dpredT = np.ascontiguousarray(
            _f8(dp8).T.reshape(2, P, slab).transpose(1, 0, 2))
        in_maps.append(
            {
                "predT8": predT,
                "dpredT8": dpredT,
                "targetT8": targetT,
                "tsub8": tsub,
                "t2b": t2bf,
                "t2subb": t2subb,
                "onesb": onesb,
                "sigma": sig,
            }
        )
    return in_maps, {"T0": T0, "v_ii": v_ii}


def kernel(pred, target, noise_sigma):
    global _NC
    from concourse.bass_utils import run_bass_kernel_spmd

    pred = np.ascontiguousarray(np.asarray(pred, dtype=np.float32))
    target = np.ascontiguousarray(np.asarray(target, dtype=np.float32))
    sig = np.asarray(noise_sigma, dtype=np.float32).reshape(1, 1)

    if _NC is None:
        _NC = _build()

    in_maps, hc = _make_in_maps(pred, target, sig)

    kw = {}
    if _TRACE:
        kw = dict(trace=True, stitch_traces=False)
    res = run_bass_kernel_spmd(_NC, in_maps, core_ids=list(range(NCORES)), **kw)
    _LAST_RESULT[0] = res

    slab = B // NCORES
    it_n = slab // P
    nv = np.float64(sig[0, 0]) ** 2
    total = 0.0
    for c, r in enumerate(res.results):
        cs = r["cs"].astype(np.float64)        # [128, 16]
        C = cs[:, :it_n]                       # [p, t] -> row c*slab + t*128 + p
        S = cs[:, it_n:]
        rows = (c * slab + np.arange(it_n)[None, :] * P
                + np.arange(P)[:, None])       # [p, t] global row ids
        v_ii = hc["v_ii"][rows]
        rowloss = (C - hc["T0"] - v_ii) / nv + np.log(S)
        total += rowloss.sum()
    loss = 2.0 * nv * (total / B)
    return np.asarray(loss, dtype=np.float32)


# revision 18
# speedup vs baseline: 5228.7087x; 1.0055x over previous
"""BMC loss (InfoNCE-style MVN loss) on 8 trn2 NeuronCores — fp8 DoubleRow.

loss = mean_i( LSE_j(l_ij) - l_ii ) * 2*sigma^2,  l_ij = (p_i.t_j - 0.5|t_j|^2)/nv
(per-row constants -0.5|p_i|^2/nv and the log-norm cancel between LSE and diag)

Device work per core (slab = 1024 pred rows, all 8192 targets):
  v'_ij = cross8_ij + t2b_j            cross8 = fp8(p).fp8(t) via DoubleRow,
      t2b = bf16(-(0.5|t_j|^2 - T0)) added into PSUM by a K=1 rank-1 matmul,
      T0 = host median offset
  C_i   = max_{j in 32Z} v'_ij + DELTA (stride-32 subsampled row max pre-pass)
  s_i   = sum_j exp((v'_ij - C_i)/nv)  (ACT reads PSUM, accum_out row sums)
returning C_i and s_i. Host (exact, f64) finishes:
  rowloss_i = (C_i - T0 - v_ii)/nv + ln s_i,  v_ii = p_i.t_i - 0.5|t_i|^2
  loss = 2*nv*mean(rowloss)

Engine plan per core — j-outer delta chain. Each 4096-col j-phase (8 PSUM
banks as 2x [128,2048] tiles) stays RESIDENT in PSUM across all 8 i-tiles:
  PE : t=0: batched K=1 bf16 rank-1s write -t2 (one ones-weight load), then
       8x K=256 fp8 DoubleRow cross(pred_0) (one DR weight load).
       t>0: one DR weight load + 8x DoubleRow adds of
       dpred_t = fp8(pred_t) - fp8(pred_{t-1}) against target, morphing
       cross(t-1) -> cross(t) in place (~37us total; every bass matmul
       self-loads weights serially, so per-tile stationary switches and
       per-i-tile rank-1s are what kill the naive layouts).
  ACT: per (t, tile): Exp over [128,2048] PSUM, scale=1/nv, bias=-C_t/nv,
       accum_out -> row sums.  ACT is the wall: 32 instrs x (1707ns exp +
       ~143ns PSUM fill + ~279ns accumulator read) ~= 68us.
  DVE: subsampled-max pre-pass + tiny fixups only.
Host: fp8/bf16 quantization, delta tiles, exact diag, final log/mean in f64.
Measured ~71us/iter (reps-differential) vs 111.6us baseline.

fp8 error budget (validated on the fixed-seed data): loss bias ~ -0.02
absolute vs tolerance ~2.7; max exp arg ~45 vs f32 overflow at 88.
"""

import numpy as np

B = 8192
D = 256
NCORES = 8
P = 128
JT = 512        # matmul moving free dim (one PSUM bank)
GROUP = 2048    # ACT instruction span (4 banks)
PHASE = 4096    # PE weight-batch span (8 banks = full PSUM)
SUBSTRIDE = 32
DELTA = 6.0


def _build(b=B, slab=B // NCORES, reps=1, variant="full"):
    import concourse.bass as bass
    import concourse.mybir as mybir
    import concourse.tile as tile
    from concourse import bacc
    from contextlib import ExitStack

    f32 = mybir.dt.float32
    f8 = mybir.dt.float8e4
    bf16 = mybir.dt.bfloat16
    DR = mybir.MatmulPerfMode.DoubleRow

    kc_n = D // P            # 2 k-tiles of 128
    it_n = slab // P         # 8 i-tiles per core
    nph = b // PHASE         # 2 phases per i-tile row
    jg = PHASE // JT         # 8 j-tiles per phase
    sub = b // SUBSTRIDE     # 256 subsampled columns

    nc = bacc.Bacc("TRN2", target_bir_lowering=False, debug=False)
    predT8 = nc.dram_tensor("predT8", [P, kc_n, slab], f8, kind="ExternalInput")
    dpredT8 = nc.dram_tensor("dpredT8", [P, kc_n, slab], f8, kind="ExternalInput")
    targetT8 = nc.dram_tensor("targetT8", [P, kc_n, b], f8, kind="ExternalInput")
    tsub8 = nc.dram_tensor("tsub8", [P, kc_n, sub], f8, kind="ExternalInput")
    t2b = nc.dram_tensor("t2b", [1, b], bf16, kind="ExternalInput")
    t2subb = nc.dram_tensor("t2subb", [1, sub], bf16, kind="ExternalInput")
    onesb = nc.dram_tensor("onesb", [1, P], bf16, kind="ExternalInput")
    sigma = nc.dram_tensor("sigma", [1, 1], f32, kind="ExternalInput")
    cs_out = nc.dram_tensor("cs", [P, 2 * it_n], f32, kind="ExternalOutput")

    with ExitStack() as ctx:
        tc = ctx.enter_context(tile.TileContext(nc))
        singles = ctx.enter_context(tc.tile_pool(name="singles", bufs=1))
        psum = ctx.enter_context(tc.tile_pool(name="psum", bufs=2, space="PSUM"))
        scratch = ctx.enter_context(tc.tile_pool(name="scratch", bufs=2))

        # ---- input DMA, round-robin across engine DGE queues ----
        issuers = [nc.sync, nc.scalar, nc.gpsimd]
        rr = [0]

        def dma(out, in_):
            eng = issuers[rr[0] % len(issuers)]
            rr[0] += 1
            eng.dma_start(out=out, in_=in_)

        predT_sb = singles.tile([P, kc_n, slab], f8)
        dma(predT_sb, predT8[:, :, :])
        dpredT_sb = singles.tile([P, kc_n, slab], f8)
        dma(dpredT_sb, dpredT8[:, :, :])
        tsub_sb = singles.tile([P, kc_n, sub], f8)
        dma(tsub_sb, tsub8[:, :, :])
        t2subb_sb = singles.tile([1, sub], bf16)
        dma(t2subb_sb, t2subb[:, :])
        t2b_sb = singles.tile([1, b], bf16)
        dma(t2b_sb, t2b[:, :])
        ones_sb = singles.tile([1, P], bf16)
        dma(ones_sb, onesb[:, :])
        sigma_sb = singles.tile([P, 1], f32)
        nc.gpsimd.dma_start(
            out=sigma_sb,
            in_=bass.AP(
                tensor=sigma[0:1, :].tensor,
                offset=sigma[0:1, :].offset,
                ap=[[0, P]] + list(sigma[0:1, :].ap[1:]),
            ),
        )
        targetT_sb = singles.tile([P, kc_n, b], f8)
        seg = 2048
        for s in range(b // seg):
            dma(targetT_sb[:, :, s * seg : (s + 1) * seg],
                targetT8[:, :, s * seg : (s + 1) * seg])

        nv128 = singles.tile([P, 1], f32)
        nc.vector.tensor_tensor(nv128, sigma_sb, sigma_sb, mybir.AluOpType.mult)
        inv128 = singles.tile([P, 1], f32)
        nc.vector.reciprocal(inv128, nv128)
        neg_inv128 = singles.tile([P, 1], f32)
        nc.vector.tensor_scalar_mul(neg_inv128, inv128, -1.0)
        # prime the ACT Exp table while DMAs run (implicit table load ~2.7us)
        warm = singles.tile([P, 1], f32)
        nc.scalar.activation(out=warm, in_=nv128,
                             func=mybir.ActivationFunctionType.Exp)

        for _rep in range(reps):
            mraw = singles.tile([P, it_n], f32)
            C_all = singles.tile([P, it_n], f32)
            bias_all = singles.tile([P, it_n], f32)
            if variant != "noexp":
                s_parts = singles.tile([P, it_n, b // GROUP], f32)
                s_all = singles.tile([P, it_n, 1], f32)

            # ---- pre-pass: subsampled row max over v' = cross8 + t2b ----
            # 4 i-tiles of [128, 256] per PSUM tile; rank-1s batched first
            # (one ones-weight load), then the 4 crosses (one LDW each).
            for half in range(it_n // 4):
                ps_pre = psum.tile([P, 4, sub], f32, tag="mm")
                for q in range(4):
                    nc.tensor.matmul(
                        out=ps_pre[:, q, :],
                        lhsT=ones_sb,
                        rhs=t2subb_sb,
                        start=True, stop=False,
                    )
                for q in range(4):
                    t = half * 4 + q
                    nc.tensor.matmul(
                        out=ps_pre[:, q, :],
                        lhsT=predT_sb[:, :, t * P : (t + 1) * P],
                        rhs=tsub_sb,
                        start=False, stop=True, perf_mode=DR,
                    )
                nc.vector.tensor_reduce(
                    out=mraw[:, half * 4 : (half + 1) * 4],
                    in_=ps_pre,
                    axis=mybir.AxisListType.X,
                    op=mybir.AluOpType.max,
                )
            nc.vector.tensor_scalar_add(C_all, mraw, DELTA)
            nc.vector.tensor_scalar_mul(bias_all, C_all, neg_inv128)

            # ---- main loop: j-outer delta chain. Each 4096-col j-phase
            # stays resident in PSUM across all 8 i-tiles: t=0 writes
            # -t2 (rank-1 batch) + cross(pred_0); t>0 adds
            # dpred_t = fp8(pred_t) - fp8(pred_{t-1}) against target,
            # morphing cross(t-1) -> cross(t) in place. One DoubleRow
            # weight load per (phase, t) instead of per 512-col tile. ----
            gpp = PHASE // GROUP  # psum tiles per phase
            jpg = GROUP // JT     # j-tiles per psum tile
            for ph in range(nph):
                pss = []
                for _g in range(gpp):
                    ps_g = psum.tile([P, GROUP], f32, tag="mm")
                    pss.append(ps_g)
                for t in range(it_n):
                    if variant == "nomm" and t == 0:
                        for g in range(gpp):
                            nc.tensor.matmul(
                                out=pss[g][:, 0:JT],
                                lhsT=dpredT_sb[:, :, 0:P],
                                rhs=targetT_sb[:, :, 0:JT],
                                start=True, stop=True, perf_mode=DR,
                            )
                    if variant != "nomm":
                        if t == 0:
                            for g in range(gpp):
                                for jj in range(jpg):
                                    j0 = ph * PHASE + g * GROUP + jj * JT
                                    nc.tensor.matmul(
                                        out=pss[g][:, jj * JT : (jj + 1) * JT],
                                        lhsT=ones_sb,
                                        rhs=t2b_sb[:, j0 : j0 + JT],
                                        start=True, stop=False,
                                    )
                        for g in range(gpp):
                            for jj in range(jpg):
                                j0 = ph * PHASE + g * GROUP + jj * JT
                                o = pss[g][:, jj * JT : (jj + 1) * JT]
                                if variant == "plaindelta":
                                    for kc in range(kc_n):
                                        nc.tensor.matmul(
                                            out=o,
                                            lhsT=dpredT_sb[
                                                :, kc, t * P : (t + 1) * P],
                                            rhs=targetT_sb[
                                                :, kc, j0 : j0 + JT],
                                            start=False,
                                            stop=(kc == kc_n - 1),
                                            skip_group_check=True,
                                        )
                                else:
                                    nc.tensor.matmul(
                                        out=o,
                                        lhsT=dpredT_sb[:, :, t * P : (t + 1) * P],
                                        rhs=targetT_sb[:, :, j0 : j0 + JT],
                                        start=False, stop=True, perf_mode=DR,
                                        skip_group_check=(t > 0),
                                    )
                    if variant != "noexp":
                        for g in range(gpp):
                            ex = scratch.tile([P, GROUP], f32, tag="ex")
                            nc.scalar.activation(
                                out=ex,
                                in_=pss[g],
                                func=mybir.ActivationFunctionType.Exp,
                                bias=bias_all[:, t : t + 1],
                                scale=inv128,
                                accum_out=s_parts[
                                    :, t, ph * gpp + g : ph * gpp + g + 1],
                            )

            if variant != "noexp":
                nc.vector.tensor_reduce(
                    out=s_all,
                    in_=s_parts,
                    axis=mybir.AxisListType.X,
                    op=mybir.AluOpType.add,
                )
                nc.sync.dma_start(out=cs_out[:, it_n : 2 * it_n], in_=s_all)
            nc.sync.dma_start(out=cs_out[:, 0:it_n], in_=C_all)

    nc.compile()
    return nc


_NC = None
_TRACE = False
_LAST_RESULT = [None]


def _f8(x):
    import ml_dtypes

    return np.asarray(x, dtype=np.float32).astype(ml_dtypes.float8_e4m3)


def _make_in_maps(pred, target, sig):
    """Shard + quantize. Returns (in_maps, host_ctx) where host_ctx has the
    exact f64 quantities the host needs to finish the loss."""
    import ml_dtypes

    slab = B // NCORES
    p64 = pred.astype(np.float64)
    t64 = target.astype(np.float64)
    t2h = 0.5 * np.sum(t64 * t64, axis=1)            # [B]
    T0 = float(np.median(t2h))
    t2bf = (-(t2h - T0)).astype(ml_dtypes.bfloat16)[None]  # [1, B]
    v_ii = np.sum(p64 * t64, axis=1) - t2h           # [B] exact diag

    pred8 = _f8(pred)    # [B, D]
    target8 = _f8(target)
    # [D, n] -> [128, 2, n] (k within tile, k-tile, column)
    targetT = np.ascontiguousarray(
        target8.T.reshape(2, P, B).transpose(1, 0, 2))
    tsub = np.ascontiguousarray(targetT[:, :, ::SUBSTRIDE])
    t2subb = np.ascontiguousarray(t2bf[:, ::SUBSTRIDE])
    onesb = np.ones((1, P), dtype=ml_dtypes.bfloat16)

    in_maps = []
    for c in range(NCORES):
        sl = slice(c * slab, (c + 1) * slab)
        p8c = pred8[sl].astype(np.float32)           # [slab, D]
        # delta tiles: row block 0 as-is; block t holds fp8(p8_t - p8_{t-1})
        dp = p8c.copy()
        dp[P:] = p8c[P:] - p8c[:-P]
        dp8 = _f8(dp).astype(np.float32)
        predT = np.ascontiguousarray(
            pred8[sl].T.reshape(2, P, slab).transpose(1, 0, 2))
        dpredT = np.ascontiguousarray(
            _f8(dp8).T.reshape(2, P, slab).transpose(1, 0, 2))
        in_maps.append(
            {
                "predT8": predT,
                "dpredT8": dpredT,
                "targetT8": targetT,
                "tsub8": tsub,
                "t2b": t2bf,
                "t2subb": t2subb,
                "onesb": onesb,
                "sigma": sig,
            }
        )
    return in_maps, {"T0": T0, "v_ii": v_ii}


def kernel(pred, target, noise_sigma):
    global _NC
    from concourse.bass_utils import run_bass_kernel_spmd

    pred = np.ascontiguousarray(np.asarray(pred, dtype=np.float32))
    target = np.ascontiguousarray(np.asarray(target, dtype=np.float32))
    sig = np.asarray(noise_sigma, dtype=np.float32).reshape(1, 1)

    if _NC is None:
        _NC = _build()

    in_maps, hc = _make_in_maps(pred, target, sig)

    kw = {}
    if _TRACE:
        kw = dict(trace=True, stitch_traces=False)
    res = run_bass_kernel_spmd(_NC, in_maps, core_ids=list(range(NCORES)), **kw)
    _LAST_RESULT[0] = res

    slab = B // NCORES
    it_n = slab // P
    nv = np.float64(sig[0, 0]) ** 2
    total = 0.0
    for c, r in enumerate(res.results):
        cs = r["cs"].astype(np.float64)        # [128, 16]
        C = cs[:, :it_n]                       # [p, t] -> row c*slab + t*128 + p
        S = cs[:, it_n:]
        rows = (c * slab + np.arange(it_n)[None, :] * P
                + np.arange(P)[:, None])       # [p, t] global row ids
        v_ii = hc["v_ii"][rows]
        rowloss = (C - hc["T0"] - v_ii) / nv + np.log(S)
        total += rowloss.sum()
    loss = 2.0 * nv * (total / B)
    return np.asarray(loss, dtype=np.float32)
